# revision 40
# baseline (speedup 1.0000x reference)
# Trainium2 Bass kernel: MultiHeadCrossAttentionLayer
#
# Sharding: data-parallel over batch. B=8 -> one batch element per NeuronCore,
# no collectives; gather = np.stack on host.
#
# Host-side prep (layout only): key/value passed pre-transposed as bf16
# [768, 4096] so no PE transposes are needed; mask passed as bf16 keep^T
# [4096, 512]; LN gamma/beta and the 1/sqrt(dh) scale folded into Wq/bq;
# v-bias+o-bias folded into cvec. k-bias dropped (softmax-invariant).
#
# Per-core computation (batch element b):
#   z    = standardize(query)             LN stats on DVE
#   qT   = Wq_eff @ z.T                   [768, 512] via PE transpose of z
#   kT   = Wk @ key.T                     [768, 4096] streamed from keyT
#   v    = (value @ Wv.T)                 [4096, 768] rows; lhsT = valT slices
#   S_T[kv, q] = kT_h.T @ qT_h            2 heads per PSUM tile; exp on ACT
#   p    = exp(S_T) * keep^T              multiplicative mask on DVE (in-place)
#   O_T  accumulated in PSUM over kv; row-sums r via ones-matmuls
#   out  = (O_T / r).T @ Wo.T + cvec
#
# Pipeline: all K-projection work and all of V-proj are interleaved into the
# attention s-loops (group 0 carries K01 tail + V, group 1 carries K23+K45)
# so the PE never idles while ACT does the exps. Attention runs at 2-head
# granularity so score PSUM tiles are 2 banks and can double-buffer.

import functools
import numpy as np

B = 8
Q = 512
KV = 4096
D = 768
H = 12
DH = 64

NT = 8            # kv chunks of 512 tokens for K/V projection staging
NSUB = KV // 128  # 32 kv sub-chunks of 128 for attention
NG = 3            # head groups


def _f32(x):
    return np.ascontiguousarray(np.asarray(x, dtype=np.float32))


def _bf16(x):
    import ml_dtypes
    return np.ascontiguousarray(np.asarray(x, dtype=np.float32).astype(ml_dtypes.bfloat16))


@functools.lru_cache(maxsize=1)
def _build():
    import concourse.bass as bass
    import concourse.tile as tile
    from concourse import bacc, mybir
    from concourse.masks import make_identity

    fp32 = mybir.dt.float32
    bf16 = mybir.dt.bfloat16
    AF = mybir.ActivationFunctionType
    ALU = mybir.AluOpType

    nc = bacc.Bacc(None, target_bir_lowering=False)

    names = {}

    with tile.TileContext(nc) as tc:
        with tc.tile_pool(name="dram", bufs=1, space="DRAM") as dram:
            d_query = dram.tile([Q, D], bf16, kind="ExternalInput")
            d_keyT = dram.tile([D, KV], bf16, kind="ExternalInput")
            d_valT = dram.tile([D, KV], bf16, kind="ExternalInput")
            d_keepT = dram.tile([KV, Q], bf16, kind="ExternalInput")
            d_wqT = dram.tile([D, D], bf16, kind="ExternalInput")
            d_wkT = dram.tile([D, D], bf16, kind="ExternalInput")
            d_wvT = dram.tile([D, D], bf16, kind="ExternalInput")
            d_woT = dram.tile([D, D], bf16, kind="ExternalInput")
            d_sel = dram.tile([2, 128, 128], bf16, kind="ExternalInput")
            d_bq = dram.tile([128, 6], fp32, kind="ExternalInput")
            d_cvec = dram.tile([1, D], fp32, kind="ExternalInput")
            d_out = dram.tile([Q, D], bf16, kind="ExternalOutput")
            d_rscratch = dram.tile([H, Q], fp32)

            names = dict(
                query=d_query.name, keyT=d_keyT.name, valT=d_valT.name,
                keepT=d_keepT.name, wqT=d_wqT.name, wkT=d_wkT.name,
                wvT=d_wvT.name, woT=d_woT.name, bq=d_bq.name, sel=d_sel.name,
                cvec=d_cvec.name, out=d_out.name,
            )

            # ---------------- persistent SBUF ----------------
            persist_cm = tc.tile_pool(name="persist", bufs=1)
            persist = persist_cm.__enter__()
            ident = persist.tile([128, 128], bf16)
            make_identity(nc, ident)
            ones_col = persist.tile([128, 1], bf16)
            nc.vector.memset(ones_col, 1.0)
            bq_sb = persist.tile([128, 6], fp32)
            nc.sync.dma_start(out=bq_sb, in_=d_bq[:])
            cvec_sb = persist.tile([128, D], fp32)
            cv_ap = d_cvec[:]
            nc.gpsimd.dma_start(
                out=cvec_sb,
                in_=bass.AP(tensor=cv_ap.tensor, offset=cv_ap.offset,
                            ap=[[0, 128]] + list(cv_ap.ap[1:])),
            )

            sel_sb = persist.tile([128, 2, 128], bf16)
            nc.sync.dma_start(out=sel_sb, in_=d_sel[:].rearrange("s p k -> p s k"))
            qT_sb = [persist.tile([128, Q], bf16, tag=f"qT{i}", name=f"qT{i}") for i in range(6)]
            oT_sb = [persist.tile([128, Q], bf16, tag=f"oT{i}", name=f"oT{i}") for i in range(6)]
            v_sb = [persist.tile([128, D], bf16, tag=f"v{i}", name=f"v{i}") for i in range(NSUB)]
            keep_sb = [persist.tile([128, Q], bf16, tag=f"kp{i}", name=f"kp{i}") for i in range(NSUB)]

            # weights (wq is scoped to phase A below)
            wpool_cm = tc.tile_pool(name="weights", bufs=1)
            wpool = wpool_cm.__enter__()
            wkT_sb = [wpool.tile([128, D], bf16, tag=f"wk{i}", name=f"wk{i}") for i in range(6)]
            wvT_sb = [wpool.tile([128, D], bf16, tag=f"wv{i}", name=f"wv{i}") for i in range(6)]
            woT_sb = [wpool.tile([128, D], bf16, tag=f"wo{i}", name=f"wo{i}") for i in range(6)]

            # kT pool: 6 logical tiles over 4 buffers (pair for group g+1 is
            # produced while pair for group g is consumed; pair g-1 is dead).
            kpool_cm = tc.tile_pool(name="kT", bufs=4)
            kpool = kpool_cm.__enter__()
            kt_tiles = [kpool.tile([128, KV], bf16, tag="kT", name=f"kT{o}")
                        for o in range(2)]

            # staging + eviction pools for the projections
            kst_cm = tc.tile_pool(name="kst", bufs=3)
            kst_pool = kst_cm.__enter__()
            vst_cm = tc.tile_pool(name="vst", bufs=2)
            vst_pool = vst_cm.__enter__()
            pj_cm = tc.tile_pool(name="ps_pj", bufs=1, space="PSUM")
            ps_pj = pj_cm.__enter__()

            # ---------- projection emitters (split DMAs across queues) ----------
            def stage_k(c):
                kst = kst_pool.tile([128, 6, 512], bf16, tag="kst")
                for i in range(6):
                    nc.sync.dma_start(
                        out=kst[:, i, :],
                        in_=d_keyT[128 * i:128 * (i + 1), 512 * c:512 * (c + 1)],
                    )
                return kst

            def emit_kproj(kst, c, o):
                # kT[o][:, 512c:512c+512] = sum_i WkT[i, o-block].T @ keyT[i, chunk]
                ps = ps_pj.tile([128, 512], fp32, tag="pj")
                for i in range(6):
                    nc.tensor.matmul(
                        out=ps[:], lhsT=wkT_sb[i][:, 128 * o:128 * (o + 1)],
                        rhs=kst[:, i, :], start=(i == 0), stop=(i == 5),
                    )
                nc.vector.tensor_copy(
                    out=kt_tiles[o][:, 512 * c:512 * (c + 1)], in_=ps[:])

            def stage_v(c):
                vst = vst_pool.tile([128, 6, 512], bf16, tag="vst")
                for i in range(6):
                    nc.sync.dma_start(
                        out=vst[:, i, :],
                        in_=d_valT[128 * i:128 * (i + 1), 512 * c:512 * (c + 1)],
                    )
                return vst

            def emit_vproj(vst, c, sub, half):
                # v[4c+sub][:, 384*half:...] = sum_i valT[i, t128].T @ WvT[i, half]
                s = 4 * c + sub
                ps = ps_pj.tile([128, 512], fp32, tag="pj")
                for i in range(6):
                    nc.tensor.matmul(
                        out=ps[:, 0:384],
                        lhsT=vst[:, i, 128 * sub:128 * (sub + 1)],
                        rhs=wvT_sb[i][:, 384 * half:384 * (half + 1)],
                        start=(i == 0), stop=(i == 5),
                    )
                nc.vector.tensor_copy(
                    out=v_sb[s][:, 384 * half:384 * (half + 1)], in_=ps[:, 0:384])

            # =============== Phase A: LN(query) -> z.T -> qT ===============
            scopeA = nc.named_scope("phaseA_ln_q"); scopeA.__enter__()
            with (
                tc.tile_pool(name="phA", bufs=2) as phA,
                tc.tile_pool(name="phA1", bufs=1) as phA1,
                tc.tile_pool(name="phA_ps", bufs=3, space="PSUM") as phA_ps,
                tc.tile_pool(name="phA_ps2", bufs=2, space="PSUM") as phA_ps2,
            ):
                # DMA priority order: query (feeds the serial LN->qT chain),
                # then wk + first K/V staging chunks, then the rest.
                q_nat = d_query[:].rearrange("(c p) d -> p c d", p=128)
                x_ts = [phA1.tile([128, D], bf16, tag=f"lnx{c}", name=f"lnx{c}") for c in range(4)]
                for c in range(4):
                    nc.sync.dma_start(out=x_ts[c][:, 0:384], in_=q_nat[:, c, 0:384])
                    nc.sync.dma_start(out=x_ts[c][:, 384:768], in_=q_nat[:, c, 384:768])
                for i in range(6):
                    nc.sync.dma_start(out=wkT_sb[i], in_=d_wkT[128 * i:128 * (i + 1), :])
                proj_state = {"budget": 0.0}
                kst0 = stage_k(0)
                vst0 = stage_v(0)
                stages_k = {0: kst0}
                stages_v = {0: vst0}
                wqT_sb = [phA1.tile([128, D], bf16, tag=f"wq{i}", name=f"wq{i}") for i in range(6)]
                for i in range(6):
                    nc.sync.dma_start(out=wqT_sb[i], in_=d_wqT[128 * i:128 * (i + 1), :])
                    nc.sync.dma_start(out=wvT_sb[i], in_=d_wvT[128 * i:128 * (i + 1), :])
                stages_k[1] = stage_k(1)
                stages_v[1] = stage_v(1)
                for s in range(NSUB):
                    nc.sync.dma_start(out=keep_sb[s], in_=d_keepT[128 * s:128 * (s + 1), :])
                for i in range(6):
                    nc.sync.dma_start(out=woT_sb[i], in_=d_woT[128 * i:128 * (i + 1), :])
                eps_sb = phA1.tile([128, 1], fp32)
                nc.vector.memset(eps_sb, 1e-5)

                z_bf = x_ts  # LN is applied in place on the staged query tiles
                for c in range(4):
                    x_t = x_ts[c]
                    # bn_stats needs free dim <= 512; use 3 subgroups of 256
                    stats = phA.tile([128, 3, 6], fp32, tag="lnst")
                    xg = x_t[:].rearrange("p (s d) -> p s d", s=3)
                    for s in range(3):
                        nc.vector.bn_stats(out=stats[:, s, :], in_=xg[:, s, :])
                    mv = phA.tile([128, 2], fp32, tag="lnmv")
                    nc.vector.bn_aggr(out=mv, in_=stats[:])
                    rstd = phA.tile([128, 1], fp32, tag="lnrs")
                    nc.scalar.activation(out=rstd, in_=mv[:, 1:2], func=AF.Sqrt,
                                         bias=eps_sb[:], scale=1.0)
                    nc.vector.reciprocal(out=rstd, in_=rstd)
                    nc.vector.tensor_scalar(
                        out=z_bf[c][:], in0=x_t[:], scalar1=mv[:, 0:1],
                        scalar2=rstd[:], op0=ALU.subtract, op1=ALU.mult,
                    )
                # transpose z -> zT (6 tiles [128, 512]); c-major so each
                # z tile's transposes start as soon as its LN completes
                zT = [phA1.tile([128, Q], bf16, tag=f"zT{i}", name=f"zT{i}") for i in range(6)]
                zps = [phA_ps.tile([128, 2, Q], bf16, tag="zTps", name=f"zps{i}") for i in range(3)]
                for c in range(4):
                    for i in range(6):
                        nc.tensor.transpose(
                            out=zps[i // 2][:, i % 2, 128 * c:128 * (c + 1)],
                            in_=z_bf[c][:, 128 * i:128 * (i + 1)],
                            identity=ident[:],
                        )
                for i in range(6):
                    nc.vector.tensor_copy(out=zT[i][:], in_=zps[i // 2][:, i % 2, :])
                # qT[o,:] = sum_i WqT[i, o-block].T @ zT[i]; group 0's two
                # o-blocks first, then the first kT chunk, then the rest
                def emit_qt(o):
                    ps = phA_ps2.tile([128, Q], fp32, tag="qps", name="qps")
                    for i in range(6):
                        nc.tensor.matmul(
                            out=ps[:], lhsT=wqT_sb[i][:, 128 * o:128 * (o + 1)],
                            rhs=zT[i][:], start=(i == 0), stop=(i == 5),
                        )
                    nc.vector.tensor_scalar(
                        out=qT_sb[o][:], in0=ps[:], scalar1=bq_sb[:, o:o + 1],
                        scalar2=None, op0=ALU.add,
                    )
                for o in range(6):
                    emit_qt(o)
            scopeA.__exit__(None, None, None)

            # =============== Prologue: kT chunk 0 of pair 0, first V chunk ===============
            scopeB = nc.named_scope("phaseB_prologue"); scopeB.__enter__()
            emit_kproj(kst0, 0, 0)
            emit_kproj(kst0, 0, 1)

            scopeB.__exit__(None, None, None)

            # deferred projection work, interleaved into attention loops.
            # Staging DMAs are issued one wave ahead of their consumers.
            from collections import deque
            pend_g = [deque(), deque(), deque()]
            kt_tiles.append(kpool.tile([128, KV], bf16, tag="kT", name="kT2"))
            kt_tiles.append(kpool.tile([128, KV], bf16, tag="kT", name="kT3"))
            kt_tiles.append(kpool.tile([128, KV], bf16, tag="kT", name="kT4"))
            kt_tiles.append(kpool.tile([128, KV], bf16, tag="kT", name="kT5"))
            # group 0: V chunk 0 first, then K01/V chunks 1..7; K23 chunk 0
            for sub in range(4):
                for half in range(2):
                    pend_g[0].append((emit_vproj, (None, 0, sub, half)))
            for c in range(1, NT):
                pend_g[0].append((emit_kproj, (None, c, 0)))
                pend_g[0].append((emit_kproj, (None, c, 1)))
                pend_g[0].append((stage_k, (c + 1 if c + 1 < NT else 0,)))
                for sub in range(4):
                    for half in range(2):
                        pend_g[0].append((emit_vproj, (None, c, sub, half)))
                if c + 1 < NT:
                    pend_g[0].append((stage_v, (c + 1,)))
            pend_g[0].append((stage_k, (1,)))
            pend_g[0].append((emit_kproj, (None, 0, 2)))
            pend_g[0].append((emit_kproj, (None, 0, 3)))
            # group 1: single pass over chunks 1..7 for all four o-blocks
            # (kT4/kT5 reuse kT0/kT1's buffers, free once group 0 is done),
            # then the re-staged chunk 0 for o=4,5.
            for c in range(1, NT):
                pend_g[1].append((emit_kproj, (None, c, 2)))
                pend_g[1].append((emit_kproj, (None, c, 3)))
                pend_g[1].append((emit_kproj, (None, c, 4)))
                pend_g[1].append((emit_kproj, (None, c, 5)))
                pend_g[1].append((stage_k, (c + 1 if c + 1 < NT else 0,)))
            pend_g[1].append((emit_kproj, (None, 0, 4)))
            pend_g[1].append((emit_kproj, (None, 0, 5)))

            # group 2: partial out-projection (i=0..3 contributions + cvec)
            eacc_holder = {}

            def emit_epartial(tc_i, half):
                ps = ps_pj.tile([128, 512], fp32, tag="pj")
                for i in range(4):
                    nc.tensor.matmul(
                        out=ps[:, 0:384],
                        lhsT=oT_sb[i][:, 128 * tc_i:128 * (tc_i + 1)],
                        rhs=woT_sb[i][:, 384 * half:384 * (half + 1)],
                        start=(i == 0), stop=(i == 3),
                    )
                nc.vector.tensor_add(
                    out=eacc_holder["t"][2 * tc_i + half][:, 0:384], in0=ps[:, 0:384],
                    in1=cvec_sb[:, 384 * half:384 * (half + 1)])

            for tc_i in range(4):
                for half in range(2):
                    pend_g[2].append((emit_epartial, (tc_i, half)))

            # per-iteration thunk quotas
            QUOTA = [len(pend_g[0]) / 27.0, len(pend_g[1]) / 21.0,
                     len(pend_g[2]) / 20.0]

            # =============== Attention (+ interleaved projections) ===============
            scopeD = nc.named_scope("phaseD_attn"); scopeD.__enter__()
            with (
                tc.tile_pool(name="phD", bufs=6) as phD,
                tc.tile_pool(name="phD1", bufs=1) as phD1,
                tc.tile_pool(name="phD_s", bufs=2, space="PSUM") as phD_s,
                tc.tile_pool(name="phD_o", bufs=2, space="PSUM") as phD_o,
                tc.tile_pool(name="phD_r", bufs=1, space="PSUM") as phD_r,
            ):
                for g in range(NG):
                    proj_state["budget"] = 2.5 if g == 1 else (2.0 if g == 0 else 0.0)
                    if g == 2:
                        # reuse the first 8 keep tiles (dead past group-2 use)
                        # as out-projection partial accumulators
                        eacc_holder["t"] = keep_sb[:8]
                    o01 = phD_o.tile([128, Q], fp32, tag="opair")
                    o23 = phD_o.tile([128, Q], fp32, tag="opair")
                    opair = (o01, o23)
                    r_ps = phD_r.tile([128, Q], fp32, tag="rps")
                    nc.vector.memset(r_ps, 1.0)
                    kts = (kt_tiles[2 * g], kt_tiles[2 * g + 1])
                    qts = (qT_sb[2 * g], qT_sb[2 * g + 1])

                    def emit_half(s, hh):
                        # scores + exp + mask for heads (4g+2hh, 4g+2hh+1)
                        sl = slice(128 * s, 128 * (s + 1))
                        s2 = phD_s.tile([128, 2, Q], fp32, tag="s4")
                        kt, qt = kts[hh], qts[hh]
                        for j in range(2):
                            nc.tensor.matmul(
                                out=s2[:, j, :],
                                lhsT=kt[64 * j:64 * (j + 1), sl],
                                rhs=qt[64 * j:64 * (j + 1), :],
                                start=True, stop=True,
                                tile_position=(64 * j, 0),
                            )
                        e2 = phD.tile([128, 2, Q], bf16, tag="e4")
                        nc.scalar.activation(out=e2[:], in_=s2[:], func=AF.Exp)
                        kap = keep_sb[s][:]
                        nc.vector.tensor_mul(
                            e2[:], e2[:],
                            bass.AP(tensor=kap.tensor, offset=kap.offset,
                                    ap=[kap.ap[0], [0, 2]] + list(kap.ap[1:])),
                        )
                        return e2

                    def emit_pv(s, epair):
                        # PV pairs first (col-tile concurrency), then the four
                        # row-sum matmuls together (4-way col-tile concurrency)
                        for hh in range(2):
                            e2 = epair[hh]
                            for j in range(2):
                                h = 2 * hh + j
                                nc.tensor.matmul(
                                    out=opair[hh][64 * j:64 * (j + 1), :],
                                    lhsT=v_sb[s][:, 256 * g + 64 * h:256 * g + 64 * (h + 1)],
                                    rhs=e2[:, j, :],
                                    start=(s == 0), stop=(s == NSUB - 1),
                                    tile_position=(0, 64 * j),
                                    skip_group_check=True,
                                )
                        for h in range(4):
                            nc.tensor.matmul(
                                out=r_ps[32 * h:32 * h + 1, :],
                                lhsT=ones_col[:],
                                rhs=epair[h // 2][:, h % 2, :],
                                start=(s == 0), stop=(s == NSUB - 1),
                                tile_position=(0, 32 * h),
                                skip_group_check=True,
                            )

                    def pop_thunks(state, g=g):
                        state["budget"] += QUOTA[g]
                        dq = pend_g[g]
                        while state["budget"] >= 1.0 and dq:
                            fn, args = dq.popleft()
                            if fn is stage_k:
                                stages_k[args[0]] = fn(*args)
                            elif fn is stage_v:
                                stages_v[args[0]] = fn(*args)
                            elif fn is emit_kproj:
                                fn(stages_k[args[1]], *args[1:])
                            elif fn is emit_vproj:
                                fn(stages_v[args[1]], *args[1:])
                            else:
                                fn(*args)
                            state["budget"] -= 1.0

                    e_q = []
                    for s in range(NSUB):
                        eA = emit_half(s, 0)
                        eB = emit_half(s, 1)
                        e_q.append((eA, eB))
                        pop_thunks(proj_state)
                        if s >= 2:
                            emit_pv(s - 2, e_q[s - 2])
                    emit_pv(NSUB - 2, e_q[NSUB - 2])
                    emit_pv(NSUB - 1, e_q[NSUB - 1])
                    # drain any leftover thunks for this group
                    proj_state["budget"] = 1e9
                    pop_thunks(proj_state)
                    proj_state["budget"] = 0.0

                    # r -> 1/r, then broadcast 1/r rows to 64-row blocks with
                    # a selector-mask matmul on the PE (no DRAM bounce):
                    # rb[p, q] = sum_c sel[c, p] * rinv[c, q]
                    rinv = phD1.tile([128, Q], fp32, tag="rinv")
                    nc.vector.reciprocal_approx_fast(out=rinv[:], in_=r_ps[:])
                    rinv_bf = phD1.tile([128, Q], bf16, tag="rinvb")
                    nc.vector.tensor_copy(out=rinv_bf[:], in_=rinv[:])
                    rb_ps = phD_s.tile([128, 2, Q], fp32, tag="s4")
                    for hh in range(2):
                        nc.tensor.matmul(
                            out=rb_ps[:, hh, :], lhsT=sel_sb[:, hh, :],
                            rhs=rinv_bf[:], start=True, stop=True,
                        )
                    nc.vector.tensor_copy(out=oT_sb[2 * g][:], in_=o01[:])
                    nc.vector.tensor_copy(out=oT_sb[2 * g + 1][:], in_=o23[:])
                    nc.vector.tensor_mul(oT_sb[2 * g][:], oT_sb[2 * g][:], rb_ps[:, 0, :])
                    nc.vector.tensor_mul(oT_sb[2 * g + 1][:], oT_sb[2 * g + 1][:], rb_ps[:, 1, :])

                # ---- Phase E tail: add oT[4:6] contributions to partials ----
                out_nat = d_out[:].rearrange("(c p) d -> p c d", p=128)
                for tchunk_i in range(4):
                    ob = phD1.tile([128, D], bf16, tag=f"ob{tchunk_i % 2}", name=f"ob{tchunk_i % 2}")
                    for half in range(2):
                        ps = phD_s.tile([128, 2, Q], fp32, tag="s4")
                        for i in range(4, 6):
                            nc.tensor.matmul(
                                out=ps[:, 0, 0:384],
                                lhsT=oT_sb[i][:, 128 * tchunk_i:128 * (tchunk_i + 1)],
                                rhs=woT_sb[i][:, 384 * half:384 * (half + 1)],
                                start=(i == 4), stop=(i == 5),
                            )
                        nc.vector.tensor_add(
                            out=ob[:, 384 * half:384 * (half + 1)], in0=ps[:, 0, 0:384],
                            in1=eacc_holder["t"][2 * tchunk_i + half][:, 0:384])
                    for qd in range(2):
                        nc.sync.dma_start(out=out_nat[:, tchunk_i, 384 * qd:384 * (qd + 1)],
                                          in_=ob[:, 384 * qd:384 * (qd + 1)])

            scopeD.__exit__(None, None, None)
            pj_cm.__exit__(None, None, None)
            vst_cm.__exit__(None, None, None)
            kst_cm.__exit__(None, None, None)
            kpool_cm.__exit__(None, None, None)
            wpool_cm.__exit__(None, None, None)
            persist_cm.__exit__(None, None, None)

    nc.compile()
    return nc, names


def kernel(**inputs):
    from concourse.bass_utils import run_bass_kernel_spmd

    nc, names, in_maps = _make_in_maps(inputs)
    res = run_bass_kernel_spmd(nc, in_maps, list(range(B)))
    out = np.stack([np.asarray(r[names["out"]], dtype=np.float32)
                    for r in res.results], axis=0)
    return out


def _make_in_maps(inputs):
    nc, names = _build()
    query = _f32(inputs["query"])
    key = _f32(inputs["key"])
    value = _f32(inputs["value"])
    mask = np.asarray(inputs["attention_mask"], dtype=np.int32)
    Wq = _f32(inputs["Wq"]); bq = _f32(inputs["bq"])
    Wk = _f32(inputs["Wk"])
    Wv = _f32(inputs["Wv"]); bv = _f32(inputs["bv"])
    Wo = _f32(inputs["Wo"]); bo = _f32(inputs["bo"])
    ln_g = _f32(inputs["ln_g"]); ln_b = _f32(inputs["ln_b"])
    scale = 1.0 / np.sqrt(DH)
    wqT = _bf16((Wq * ln_g[None, :] * scale).T)
    bq_eff = (ln_b @ Wq.T + bq) * scale
    bq_arr = _f32(bq_eff.reshape(6, 128).T)
    wkT = _bf16(Wk.T)
    wvT = _bf16(Wv.T)
    woT = _bf16(Wo.T)
    cvec = _f32((bv @ Wo.T + bo).reshape(1, D))
    sel = np.zeros((2, 128, 128), np.float32)
    sel[0, 0, 0:64] = 1.0
    sel[0, 32, 64:128] = 1.0
    sel[1, 64, 0:64] = 1.0
    sel[1, 96, 64:128] = 1.0
    keep = (1 - mask).astype(np.float32)
    in_maps = []
    for b in range(B):
        in_maps.append({
            names["query"]: _bf16(query[b]),
            names["keyT"]: _bf16(key[b].T),
            names["valT"]: _bf16(value[b].T),
            names["keepT"]: _bf16(keep[b].T),
            names["wqT"]: wqT, names["wkT"]: wkT, names["wvT"]: wvT,
            names["woT"]: woT, names["bq"]: bq_arr, names["cvec"]: cvec,
            names["sel"]: _bf16(sel),
        })
    return nc, names, in_maps


def run_traced(**inputs):
    """Run with tracing enabled; returns exec_time_ns (or None)."""
    from concourse.bass_utils import run_bass_kernel_spmd
    nc, names, in_maps = _make_in_maps(inputs)
    res = run_bass_kernel_spmd(nc, in_maps, list(range(B)), trace=True)
    if res.instructions_and_trace is not None:
        print("trace:", res.instructions_and_trace[1])
    print("mean exec ns:", res.mean_exec_time_ns, "max core:", res.max_exec_time_core_id)
    if res.per_core_scope_times:
        for scope, cores in sorted(res.per_core_scope_times.items()):
            for cid, dur in cores.items():
                print(f"  scope {scope}: core{cid} {dur} ns")
    return res.exec_time_ns


if __name__ == "__main__":
    rng = np.random.default_rng(0)
    dummy = {
        "query": rng.standard_normal((B, Q, D), dtype=np.float32),
        "key": rng.standard_normal((B, KV, D), dtype=np.float32),
        "value": rng.standard_normal((B, KV, D), dtype=np.float32),
        "attention_mask": rng.integers(0, 2, (B, Q, KV)).astype(np.int32),
        "Wq": rng.standard_normal((D, D), dtype=np.float32) / 27.7,
        "bq": np.zeros(D, np.float32),
        "Wk": rng.standard_normal((D, D), dtype=np.float32) / 27.7,
        "bk": np.zeros(D, np.float32),
        "Wv": rng.standard_normal((D, D), dtype=np.float32) / 27.7,
        "bv": np.zeros(D, np.float32),
        "Wo": rng.standard_normal((D, D), dtype=np.float32) / 27.7,
        "bo": np.zeros(D, np.float32),
        "ln_g": np.ones(D, np.float32),
        "ln_b": np.zeros(D, np.float32),
    }
    out = kernel(**dummy)
    print("out", out.shape, out.dtype, float(np.abs(out).mean()))


# revision 41
# speedup vs baseline: 1.0151x; 1.0151x over previous
# Trainium2 Bass kernel: MultiHeadCrossAttentionLayer
#
# Sharding: data-parallel over batch. B=8 -> one batch element per NeuronCore,
# no collectives; gather = np.stack on host.
#
# Host-side prep (layout only): key/value passed pre-transposed as bf16
# [768, 4096] so no PE transposes are needed; mask passed as bf16 keep^T
# [4096, 512]; LN gamma/beta and the 1/sqrt(dh) scale folded into Wq/bq;
# v-bias+o-bias folded into cvec. k-bias dropped (softmax-invariant).
#
# Per-core computation (batch element b):
#   z    = standardize(query)             LN stats on DVE
#   qT   = Wq_eff @ z.T                   [768, 512] via PE transpose of z
#   kT   = Wk @ key.T                     [768, 4096] streamed from keyT
#   v    = (value @ Wv.T)                 [4096, 768] rows; lhsT = valT slices
#   S_T[kv, q] = kT_h.T @ qT_h            2 heads per PSUM tile; exp on ACT
#   p    = exp(S_T) * keep^T              multiplicative mask on DVE (in-place)
#   O_T  accumulated in PSUM over kv; row-sums r via ones-matmuls
#   out  = (O_T / r).T @ Wo.T + cvec
#
# Pipeline: all K-projection work and all of V-proj are interleaved into the
# attention s-loops (group 0 carries K01 tail + V, group 1 carries K23+K45)
# so the PE never idles while ACT does the exps. Attention runs at 2-head
# granularity so score PSUM tiles are 2 banks and can double-buffer.

import functools
import numpy as np

B = 8
Q = 512
KV = 4096
D = 768
H = 12
DH = 64

NT = 8            # kv chunks of 512 tokens for K/V projection staging
NSUB = KV // 128  # 32 kv sub-chunks of 128 for attention
NG = 3            # head groups


def _f32(x):
    return np.ascontiguousarray(np.asarray(x, dtype=np.float32))


def _bf16(x):
    import ml_dtypes
    return np.ascontiguousarray(np.asarray(x, dtype=np.float32).astype(ml_dtypes.bfloat16))


@functools.lru_cache(maxsize=1)
def _build():
    import concourse.bass as bass
    import concourse.tile as tile
    from concourse import bacc, mybir
    from concourse.masks import make_identity

    fp32 = mybir.dt.float32
    bf16 = mybir.dt.bfloat16
    AF = mybir.ActivationFunctionType
    ALU = mybir.AluOpType

    nc = bacc.Bacc(None, target_bir_lowering=False)

    names = {}

    with tile.TileContext(nc) as tc:
        with tc.tile_pool(name="dram", bufs=1, space="DRAM") as dram:
            d_query = dram.tile([Q, D], bf16, kind="ExternalInput")
            d_keyT = dram.tile([D, KV], bf16, kind="ExternalInput")
            d_valT = dram.tile([D, KV], bf16, kind="ExternalInput")
            d_keepT = dram.tile([KV, Q], bf16, kind="ExternalInput")
            d_wqT = dram.tile([D, D], bf16, kind="ExternalInput")
            d_wkT = dram.tile([D, D], bf16, kind="ExternalInput")
            d_wvT = dram.tile([D, D], bf16, kind="ExternalInput")
            d_woT = dram.tile([D, D], bf16, kind="ExternalInput")
            d_sel = dram.tile([2, 128, 128], bf16, kind="ExternalInput")
            d_bq = dram.tile([128, 6], fp32, kind="ExternalInput")
            d_cvec = dram.tile([1, D], fp32, kind="ExternalInput")
            d_out = dram.tile([Q, D], bf16, kind="ExternalOutput")
            d_rscratch = dram.tile([H, Q], fp32)

            names = dict(
                query=d_query.name, keyT=d_keyT.name, valT=d_valT.name,
                keepT=d_keepT.name, wqT=d_wqT.name, wkT=d_wkT.name,
                wvT=d_wvT.name, woT=d_woT.name, bq=d_bq.name, sel=d_sel.name,
                cvec=d_cvec.name, out=d_out.name,
            )

            # ---------------- persistent SBUF ----------------
            persist_cm = tc.tile_pool(name="persist", bufs=1)
            persist = persist_cm.__enter__()
            ident = persist.tile([128, 128], bf16)
            make_identity(nc, ident)
            ones_col = persist.tile([128, 1], bf16)
            nc.vector.memset(ones_col, 1.0)
            bq_sb = persist.tile([128, 6], fp32)
            nc.sync.dma_start(out=bq_sb, in_=d_bq[:])
            cvec_sb = persist.tile([128, D], fp32)
            cv_ap = d_cvec[:]
            nc.gpsimd.dma_start(
                out=cvec_sb,
                in_=bass.AP(tensor=cv_ap.tensor, offset=cv_ap.offset,
                            ap=[[0, 128]] + list(cv_ap.ap[1:])),
            )

            sel_sb = persist.tile([128, 2, 128], bf16)
            nc.sync.dma_start(out=sel_sb, in_=d_sel[:].rearrange("s p k -> p s k"))
            qT_sb = [persist.tile([128, Q], bf16, tag=f"qT{i}", name=f"qT{i}") for i in range(6)]
            oT_sb = [persist.tile([128, Q], bf16, tag=f"oT{i}", name=f"oT{i}") for i in range(6)]
            v_sb = [persist.tile([128, D], bf16, tag=f"v{i}", name=f"v{i}") for i in range(NSUB)]
            keep_sb = [persist.tile([128, Q], bf16, tag=f"kp{i}", name=f"kp{i}") for i in range(NSUB)]

            # weights (wq is scoped to phase A below)
            wpool_cm = tc.tile_pool(name="weights", bufs=1)
            wpool = wpool_cm.__enter__()
            wkT_sb = [wpool.tile([128, D], bf16, tag=f"wk{i}", name=f"wk{i}") for i in range(6)]
            wvT_sb = [wpool.tile([128, D], bf16, tag=f"wv{i}", name=f"wv{i}") for i in range(6)]
            woT_sb = [wpool.tile([128, D], bf16, tag=f"wo{i}", name=f"wo{i}") for i in range(6)]

            # kT pool: 6 logical tiles over 4 buffers (pair for group g+1 is
            # produced while pair for group g is consumed; pair g-1 is dead).
            kpool_cm = tc.tile_pool(name="kT", bufs=4)
            kpool = kpool_cm.__enter__()
            kt_tiles = [kpool.tile([128, KV], bf16, tag="kT", name=f"kT{o}")
                        for o in range(2)]

            # staging + eviction pools for the projections
            kst_cm = tc.tile_pool(name="kst", bufs=3)
            kst_pool = kst_cm.__enter__()
            vst_cm = tc.tile_pool(name="vst", bufs=2)
            vst_pool = vst_cm.__enter__()
            pj_cm = tc.tile_pool(name="ps_pj", bufs=1, space="PSUM")
            ps_pj = pj_cm.__enter__()

            # ---------- projection emitters (split DMAs across queues) ----------
            def stage_k(c):
                kst = kst_pool.tile([128, 6, 512], bf16, tag="kst")
                for i in range(6):
                    nc.sync.dma_start(
                        out=kst[:, i, :],
                        in_=d_keyT[128 * i:128 * (i + 1), 512 * c:512 * (c + 1)],
                    )
                return kst

            def emit_kproj(kst, c, o):
                # kT[o][:, 512c:512c+512] = sum_i WkT[i, o-block].T @ keyT[i, chunk]
                ps = ps_pj.tile([128, 512], fp32, tag="pj")
                for i in range(6):
                    nc.tensor.matmul(
                        out=ps[:], lhsT=wkT_sb[i][:, 128 * o:128 * (o + 1)],
                        rhs=kst[:, i, :], start=(i == 0), stop=(i == 5),
                    )
                nc.vector.tensor_copy(
                    out=kt_tiles[o][:, 512 * c:512 * (c + 1)], in_=ps[:])

            def stage_v(c):
                vst = vst_pool.tile([128, 6, 512], bf16, tag="vst")
                for i in range(6):
                    nc.sync.dma_start(
                        out=vst[:, i, :],
                        in_=d_valT[128 * i:128 * (i + 1), 512 * c:512 * (c + 1)],
                    )
                return vst

            def emit_vproj(vst, c, sub, half):
                # v[4c+sub][:, 384*half:...] = sum_i valT[i, t128].T @ WvT[i, half]
                s = 4 * c + sub
                ps = ps_pj.tile([128, 512], fp32, tag="pj")
                for i in range(6):
                    nc.tensor.matmul(
                        out=ps[:, 0:384],
                        lhsT=vst[:, i, 128 * sub:128 * (sub + 1)],
                        rhs=wvT_sb[i][:, 384 * half:384 * (half + 1)],
                        start=(i == 0), stop=(i == 5),
                    )
                nc.vector.tensor_copy(
                    out=v_sb[s][:, 384 * half:384 * (half + 1)], in_=ps[:, 0:384])

            # =============== Phase A: LN(query) -> z.T -> qT ===============
            scopeA = nc.named_scope("phaseA_ln_q"); scopeA.__enter__()
            with (
                tc.tile_pool(name="phA", bufs=2) as phA,
                tc.tile_pool(name="phA1", bufs=1) as phA1,
                tc.tile_pool(name="phA_ps", bufs=3, space="PSUM") as phA_ps,
                tc.tile_pool(name="phA_ps2", bufs=2, space="PSUM") as phA_ps2,
            ):
                # DMA priority order: query (feeds the serial LN->qT chain),
                # then wk + first K/V staging chunks, then the rest.
                q_nat = d_query[:].rearrange("(c p) d -> p c d", p=128)
                x_ts = [phA1.tile([128, D], bf16, tag=f"lnx{c}", name=f"lnx{c}") for c in range(4)]
                for c in range(4):
                    nc.sync.dma_start(out=x_ts[c][:, 0:384], in_=q_nat[:, c, 0:384])
                    nc.sync.dma_start(out=x_ts[c][:, 384:768], in_=q_nat[:, c, 384:768])
                for i in range(6):
                    nc.sync.dma_start(out=wkT_sb[i], in_=d_wkT[128 * i:128 * (i + 1), :])
                proj_state = {"budget": 0.0}
                kst0 = stage_k(0)
                vst0 = stage_v(0)
                stages_k = {0: kst0}
                stages_v = {0: vst0}
                wqT_sb = [phA1.tile([128, D], bf16, tag=f"wq{i}", name=f"wq{i}") for i in range(6)]
                for i in range(6):
                    nc.sync.dma_start(out=wqT_sb[i], in_=d_wqT[128 * i:128 * (i + 1), :])
                    nc.sync.dma_start(out=wvT_sb[i], in_=d_wvT[128 * i:128 * (i + 1), :])
                stages_k[1] = stage_k(1)
                stages_v[1] = stage_v(1)
                for s in range(NSUB):
                    nc.sync.dma_start(out=keep_sb[s], in_=d_keepT[128 * s:128 * (s + 1), :])
                for i in range(6):
                    nc.sync.dma_start(out=woT_sb[i], in_=d_woT[128 * i:128 * (i + 1), :])
                eps_sb = phA1.tile([128, 1], fp32)
                nc.vector.memset(eps_sb, 1e-5)

                z_bf = x_ts  # LN is applied in place on the staged query tiles
                for c in range(4):
                    x_t = x_ts[c]
                    # bn_stats needs free dim <= 512; use 3 subgroups of 256
                    stats = phA.tile([128, 3, 6], fp32, tag="lnst")
                    xg = x_t[:].rearrange("p (s d) -> p s d", s=3)
                    for s in range(3):
                        nc.vector.bn_stats(out=stats[:, s, :], in_=xg[:, s, :])
                    mv = phA.tile([128, 2], fp32, tag="lnmv")
                    nc.vector.bn_aggr(out=mv, in_=stats[:])
                    rstd = phA.tile([128, 1], fp32, tag="lnrs")
                    nc.scalar.activation(out=rstd, in_=mv[:, 1:2], func=AF.Sqrt,
                                         bias=eps_sb[:], scale=1.0)
                    nc.vector.reciprocal(out=rstd, in_=rstd)
                    nc.vector.tensor_scalar(
                        out=z_bf[c][:], in0=x_t[:], scalar1=mv[:, 0:1],
                        scalar2=rstd[:], op0=ALU.subtract, op1=ALU.mult,
                    )
                # transpose z -> zT (6 tiles [128, 512]); c-major so each
                # z tile's transposes start as soon as its LN completes
                zT = [phA1.tile([128, Q], bf16, tag=f"zT{i}", name=f"zT{i}") for i in range(6)]
                zps = [phA_ps.tile([128, 2, Q], bf16, tag="zTps", name=f"zps{i}") for i in range(3)]
                for c in range(4):
                    for i in range(6):
                        nc.tensor.transpose(
                            out=zps[i // 2][:, i % 2, 128 * c:128 * (c + 1)],
                            in_=z_bf[c][:, 128 * i:128 * (i + 1)],
                            identity=ident[:],
                        )
                for i in range(6):
                    nc.vector.tensor_copy(out=zT[i][:], in_=zps[i // 2][:, i % 2, :])
                # qT[o,:] = sum_i WqT[i, o-block].T @ zT[i]; group 0's two
                # o-blocks first, then the first kT chunk, then the rest
                def emit_qt(o):
                    ps = phA_ps2.tile([128, Q], fp32, tag="qps", name="qps")
                    for i in range(6):
                        nc.tensor.matmul(
                            out=ps[:], lhsT=wqT_sb[i][:, 128 * o:128 * (o + 1)],
                            rhs=zT[i][:], start=(i == 0), stop=(i == 5),
                        )
                    nc.vector.tensor_scalar(
                        out=qT_sb[o][:], in0=ps[:], scalar1=bq_sb[:, o:o + 1],
                        scalar2=None, op0=ALU.add,
                    )
                for o in range(6):
                    emit_qt(o)
            scopeA.__exit__(None, None, None)

            # =============== Prologue: kT chunk 0 of pair 0, first V chunk ===============
            scopeB = nc.named_scope("phaseB_prologue"); scopeB.__enter__()
            emit_kproj(kst0, 0, 0)
            emit_kproj(kst0, 0, 1)

            scopeB.__exit__(None, None, None)

            # deferred projection work, interleaved into attention loops.
            # Staging DMAs are issued one wave ahead of their consumers.
            from collections import deque
            pend_g = [deque(), deque(), deque()]
            kt_tiles.append(kpool.tile([128, KV], bf16, tag="kT", name="kT2"))
            kt_tiles.append(kpool.tile([128, KV], bf16, tag="kT", name="kT3"))
            kt_tiles.append(kpool.tile([128, KV], bf16, tag="kT", name="kT4"))
            kt_tiles.append(kpool.tile([128, KV], bf16, tag="kT", name="kT5"))
            # group 0: V chunk 0 first, then K01/V chunks 1..7; K23 chunk 0
            for sub in range(4):
                for half in range(2):
                    pend_g[0].append((emit_vproj, (None, 0, sub, half)))
            for c in range(1, NT):
                pend_g[0].append((emit_kproj, (None, c, 0)))
                pend_g[0].append((emit_kproj, (None, c, 1)))
                pend_g[0].append((stage_k, (c + 1 if c + 1 < NT else 0,)))
                for sub in range(4):
                    for half in range(2):
                        pend_g[0].append((emit_vproj, (None, c, sub, half)))
                if c + 1 < NT:
                    pend_g[0].append((stage_v, (c + 1,)))
            pend_g[0].append((stage_k, (1,)))
            pend_g[0].append((emit_kproj, (None, 0, 2)))
            pend_g[0].append((emit_kproj, (None, 0, 3)))
            # group 1: single pass over chunks 1..7 for all four o-blocks
            # (kT4/kT5 reuse kT0/kT1's buffers, free once group 0 is done),
            # then the re-staged chunk 0 for o=4,5.
            for c in range(1, NT):
                pend_g[1].append((emit_kproj, (None, c, 2)))
                pend_g[1].append((emit_kproj, (None, c, 3)))
                pend_g[1].append((emit_kproj, (None, c, 4)))
                pend_g[1].append((emit_kproj, (None, c, 5)))
                pend_g[1].append((stage_k, (c + 1 if c + 1 < NT else 0,)))
            pend_g[1].append((emit_kproj, (None, 0, 4)))
            pend_g[1].append((emit_kproj, (None, 0, 5)))

            # group 2: partial out-projection (i=0..3 contributions + cvec)
            eacc_holder = {}

            def emit_epartial(tc_i, half):
                ps = ps_pj.tile([128, 512], fp32, tag="pj")
                for i in range(4):
                    nc.tensor.matmul(
                        out=ps[:, 0:384],
                        lhsT=oT_sb[i][:, 128 * tc_i:128 * (tc_i + 1)],
                        rhs=woT_sb[i][:, 384 * half:384 * (half + 1)],
                        start=(i == 0), stop=(i == 3),
                    )
                nc.vector.tensor_add(
                    out=eacc_holder["t"][2 * tc_i + half][:, 0:384], in0=ps[:, 0:384],
                    in1=cvec_sb[:, 384 * half:384 * (half + 1)])

            for tc_i in range(4):
                for half in range(2):
                    pend_g[2].append((emit_epartial, (tc_i, half)))

            # per-iteration thunk quotas
            QUOTA = [len(pend_g[0]) / 27.0, len(pend_g[1]) / 26.0,
                     len(pend_g[2]) / 20.0]

            # =============== Attention (+ interleaved projections) ===============
            scopeD = nc.named_scope("phaseD_attn"); scopeD.__enter__()
            with (
                tc.tile_pool(name="phD", bufs=6) as phD,
                tc.tile_pool(name="phD1", bufs=1) as phD1,
                tc.tile_pool(name="phD_s", bufs=2, space="PSUM") as phD_s,
                tc.tile_pool(name="phD_o", bufs=2, space="PSUM") as phD_o,
                tc.tile_pool(name="phD_r", bufs=1, space="PSUM") as phD_r,
            ):
                for g in range(NG):
                    proj_state["budget"] = 2.5 if g == 1 else (2.0 if g == 0 else 0.0)
                    if g == 2:
                        # reuse the first 8 keep tiles (dead past group-2 use)
                        # as out-projection partial accumulators
                        eacc_holder["t"] = keep_sb[:8]
                    o01 = phD_o.tile([128, Q], fp32, tag="opair")
                    o23 = phD_o.tile([128, Q], fp32, tag="opair")
                    opair = (o01, o23)
                    r_ps = phD_r.tile([128, Q], fp32, tag="rps")
                    nc.vector.memset(r_ps, 1.0)
                    kts = (kt_tiles[2 * g], kt_tiles[2 * g + 1])
                    qts = (qT_sb[2 * g], qT_sb[2 * g + 1])

                    def emit_half(s, hh):
                        # scores + exp + mask for heads (4g+2hh, 4g+2hh+1)
                        sl = slice(128 * s, 128 * (s + 1))
                        s2 = phD_s.tile([128, 2, Q], fp32, tag="s4")
                        kt, qt = kts[hh], qts[hh]
                        for j in range(2):
                            nc.tensor.matmul(
                                out=s2[:, j, :],
                                lhsT=kt[64 * j:64 * (j + 1), sl],
                                rhs=qt[64 * j:64 * (j + 1), :],
                                start=True, stop=True,
                                tile_position=(64 * j, 0),
                            )
                        e2 = phD.tile([128, 2, Q], bf16, tag="e4")
                        nc.scalar.activation(out=e2[:], in_=s2[:], func=AF.Exp)
                        kap = keep_sb[s][:]
                        nc.vector.tensor_mul(
                            e2[:], e2[:],
                            bass.AP(tensor=kap.tensor, offset=kap.offset,
                                    ap=[kap.ap[0], [0, 2]] + list(kap.ap[1:])),
                        )
                        return e2

                    def emit_pv(s, epair):
                        # PV pairs first (col-tile concurrency), then the four
                        # row-sum matmuls together (4-way col-tile concurrency)
                        for hh in range(2):
                            e2 = epair[hh]
                            for j in range(2):
                                h = 2 * hh + j
                                nc.tensor.matmul(
                                    out=opair[hh][64 * j:64 * (j + 1), :],
                                    lhsT=v_sb[s][:, 256 * g + 64 * h:256 * g + 64 * (h + 1)],
                                    rhs=e2[:, j, :],
                                    start=(s == 0), stop=(s == NSUB - 1),
                                    tile_position=(0, 64 * j),
                                    skip_group_check=True,
                                )
                        for h in range(4):
                            nc.tensor.matmul(
                                out=r_ps[32 * h:32 * h + 1, :],
                                lhsT=ones_col[:],
                                rhs=epair[h // 2][:, h % 2, :],
                                start=(s == 0), stop=(s == NSUB - 1),
                                tile_position=(0, 32 * h),
                                skip_group_check=True,
                            )

                    def pop_thunks(state, g=g):
                        state["budget"] += QUOTA[g]
                        dq = pend_g[g]
                        while state["budget"] >= 1.0 and dq:
                            fn, args = dq.popleft()
                            if fn is stage_k:
                                stages_k[args[0]] = fn(*args)
                            elif fn is stage_v:
                                stages_v[args[0]] = fn(*args)
                            elif fn is emit_kproj:
                                fn(stages_k[args[1]], *args[1:])
                            elif fn is emit_vproj:
                                fn(stages_v[args[1]], *args[1:])
                            else:
                                fn(*args)
                            state["budget"] -= 1.0

                    e_q = []
                    for s in range(NSUB):
                        eA = emit_half(s, 0)
                        eB = emit_half(s, 1)
                        e_q.append((eA, eB))
                        pop_thunks(proj_state)
                        if s >= 2:
                            emit_pv(s - 2, e_q[s - 2])
                    emit_pv(NSUB - 2, e_q[NSUB - 2])
                    emit_pv(NSUB - 1, e_q[NSUB - 1])
                    # drain any leftover thunks for this group
                    proj_state["budget"] = 1e9
                    pop_thunks(proj_state)
                    proj_state["budget"] = 0.0

                    # r -> 1/r, then broadcast 1/r rows to 64-row blocks with
                    # a selector-mask matmul on the PE (no DRAM bounce):
                    # rb[p, q] = sum_c sel[c, p] * rinv[c, q]
                    rinv = phD1.tile([128, Q], fp32, tag="rinv")
                    nc.vector.reciprocal_approx_fast(out=rinv[:], in_=r_ps[:])
                    rinv_bf = phD1.tile([128, Q], bf16, tag="rinvb")
                    nc.vector.tensor_copy(out=rinv_bf[:], in_=rinv[:])
                    rb_ps = phD_s.tile([128, 2, Q], fp32, tag="s4")
                    for hh in range(2):
                        nc.tensor.matmul(
                            out=rb_ps[:, hh, :], lhsT=sel_sb[:, hh, :],
                            rhs=rinv_bf[:], start=True, stop=True,
                        )
                    nc.vector.tensor_copy(out=oT_sb[2 * g][:], in_=o01[:])
                    nc.vector.tensor_copy(out=oT_sb[2 * g + 1][:], in_=o23[:])
                    nc.vector.tensor_mul(oT_sb[2 * g][:], oT_sb[2 * g][:], rb_ps[:, 0, :])
                    nc.vector.tensor_mul(oT_sb[2 * g + 1][:], oT_sb[2 * g + 1][:], rb_ps[:, 1, :])

                # ---- Phase E tail: add oT[4:6] contributions to partials ----
                out_nat = d_out[:].rearrange("(c p) d -> p c d", p=128)
                for tchunk_i in range(4):
                    ob = phD1.tile([128, D], bf16, tag=f"ob{tchunk_i % 2}", name=f"ob{tchunk_i % 2}")
                    for half in range(2):
                        ps = phD_s.tile([128, 2, Q], fp32, tag="s4")
                        for i in range(4, 6):
                            nc.tensor.matmul(
                                out=ps[:, 0, 0:384],
                                lhsT=oT_sb[i][:, 128 * tchunk_i:128 * (tchunk_i + 1)],
                                rhs=woT_sb[i][:, 384 * half:384 * (half + 1)],
                                start=(i == 4), stop=(i == 5),
                            )
                        nc.vector.tensor_add(
                            out=ob[:, 384 * half:384 * (half + 1)], in0=ps[:, 0, 0:384],
                            in1=eacc_holder["t"][2 * tchunk_i + half][:, 0:384])
                    for qd in range(2):
                        nc.sync.dma_start(out=out_nat[:, tchunk_i, 384 * qd:384 * (qd + 1)],
                                          in_=ob[:, 384 * qd:384 * (qd + 1)])

            scopeD.__exit__(None, None, None)
            pj_cm.__exit__(None, None, None)
            vst_cm.__exit__(None, None, None)
            kst_cm.__exit__(None, None, None)
            kpool_cm.__exit__(None, None, None)
            wpool_cm.__exit__(None, None, None)
            persist_cm.__exit__(None, None, None)

    nc.compile()
    return nc, names


def kernel(**inputs):
    from concourse.bass_utils import run_bass_kernel_spmd

    nc, names, in_maps = _make_in_maps(inputs)
    res = run_bass_kernel_spmd(nc, in_maps, list(range(B)))
    out = np.stack([np.asarray(r[names["out"]], dtype=np.float32)
                    for r in res.results], axis=0)
    return out


def _make_in_maps(inputs):
    nc, names = _build()
    query = _f32(inputs["query"])
    key = _f32(inputs["key"])
    value = _f32(inputs["value"])
    mask = np.asarray(inputs["attention_mask"], dtype=np.int32)
    Wq = _f32(inputs["Wq"]); bq = _f32(inputs["bq"])
    Wk = _f32(inputs["Wk"])
    Wv = _f32(inputs["Wv"]); bv = _f32(inputs["bv"])
    Wo = _f32(inputs["Wo"]); bo = _f32(inputs["bo"])
    ln_g = _f32(inputs["ln_g"]); ln_b = _f32(inputs["ln_b"])
    scale = 1.0 / np.sqrt(DH)
    wqT = _bf16((Wq * ln_g[None, :] * scale).T)
    bq_eff = (ln_b @ Wq.T + bq) * scale
    bq_arr = _f32(bq_eff.reshape(6, 128).T)
    wkT = _bf16(Wk.T)
    wvT = _bf16(Wv.T)
    woT = _bf16(Wo.T)
    cvec = _f32((bv @ Wo.T + bo).reshape(1, D))
    sel = np.zeros((2, 128, 128), np.float32)
    sel[0, 0, 0:64] = 1.0
    sel[0, 32, 64:128] = 1.0
    sel[1, 64, 0:64] = 1.0
    sel[1, 96, 64:128] = 1.0
    keep = (1 - mask).astype(np.float32)
    in_maps = []
    for b in range(B):
        in_maps.append({
            names["query"]: _bf16(query[b]),
            names["keyT"]: _bf16(key[b].T),
            names["valT"]: _bf16(value[b].T),
            names["keepT"]: _bf16(keep[b].T),
            names["wqT"]: wqT, names["wkT"]: wkT, names["wvT"]: wvT,
            names["woT"]: woT, names["bq"]: bq_arr, names["cvec"]: cvec,
            names["sel"]: _bf16(sel),
        })
    return nc, names, in_maps


def run_traced(**inputs):
    """Run with tracing enabled; returns exec_time_ns (or None)."""
    from concourse.bass_utils import run_bass_kernel_spmd
    nc, names, in_maps = _make_in_maps(inputs)
    res = run_bass_kernel_spmd(nc, in_maps, list(range(B)), trace=True)
    if res.instructions_and_trace is not None:
        print("trace:", res.instructions_and_trace[1])
    print("mean exec ns:", res.mean_exec_time_ns, "max core:", res.max_exec_time_core_id)
    if res.per_core_scope_times:
        for scope, cores in sorted(res.per_core_scope_times.items()):
            for cid, dur in cores.items():
                print(f"  scope {scope}: core{cid} {dur} ns")
    return res.exec_time_ns


if __name__ == "__main__":
    rng = np.random.default_rng(0)
    dummy = {
        "query": rng.standard_normal((B, Q, D), dtype=np.float32),
        "key": rng.standard_normal((B, KV, D), dtype=np.float32),
        "value": rng.standard_normal((B, KV, D), dtype=np.float32),
        "attention_mask": rng.integers(0, 2, (B, Q, KV)).astype(np.int32),
        "Wq": rng.standard_normal((D, D), dtype=np.float32) / 27.7,
        "bq": np.zeros(D, np.float32),
        "Wk": rng.standard_normal((D, D), dtype=np.float32) / 27.7,
        "bk": np.zeros(D, np.float32),
        "Wv": rng.standard_normal((D, D), dtype=np.float32) / 27.7,
        "bv": np.zeros(D, np.float32),
        "Wo": rng.standard_normal((D, D), dtype=np.float32) / 27.7,
        "bo": np.zeros(D, np.float32),
        "ln_g": np.ones(D, np.float32),
        "ln_b": np.zeros(D, np.float32),
    }
    out = kernel(**dummy)
    print("out", out.shape, out.dtype, float(np.abs(out).mean()))


# revision 42
# speedup vs baseline: 1.0193x; 1.0041x over previous
# Trainium2 Bass kernel: MultiHeadCrossAttentionLayer
#
# Sharding: data-parallel over batch. B=8 -> one batch element per NeuronCore,
# no collectives; gather = np.stack on host.
#
# Host-side prep (layout only): key/value passed pre-transposed as bf16
# [768, 4096] so no PE transposes are needed; mask passed as bf16 keep^T
# [4096, 512]; LN gamma/beta and the 1/sqrt(dh) scale folded into Wq/bq;
# v-bias+o-bias folded into cvec. k-bias dropped (softmax-invariant).
#
# Per-core computation (batch element b):
#   z    = standardize(query)             LN stats on DVE
#   qT   = Wq_eff @ z.T                   [768, 512] via PE transpose of z
#   kT   = Wk @ key.T                     [768, 4096] streamed from keyT
#   v    = (value @ Wv.T)                 [4096, 768] rows; lhsT = valT slices
#   S_T[kv, q] = kT_h.T @ qT_h            2 heads per PSUM tile; exp on ACT
#   p    = exp(S_T) * keep^T              multiplicative mask on DVE (in-place)
#   O_T  accumulated in PSUM over kv; row-sums r via ones-matmuls
#   out  = (O_T / r).T @ Wo.T + cvec
#
# Pipeline: all K-projection work and all of V-proj are interleaved into the
# attention s-loops (group 0 carries K01 tail + V, group 1 carries K23+K45)
# so the PE never idles while ACT does the exps. Attention runs at 2-head
# granularity so score PSUM tiles are 2 banks and can double-buffer.

import functools
import numpy as np

B = 8
Q = 512
KV = 4096
D = 768
H = 12
DH = 64

NT = 8            # kv chunks of 512 tokens for K/V projection staging
NSUB = KV // 128  # 32 kv sub-chunks of 128 for attention
NG = 3            # head groups


def _f32(x):
    return np.ascontiguousarray(np.asarray(x, dtype=np.float32))


def _bf16(x):
    import ml_dtypes
    return np.ascontiguousarray(np.asarray(x, dtype=np.float32).astype(ml_dtypes.bfloat16))


@functools.lru_cache(maxsize=1)
def _build():
    import concourse.bass as bass
    import concourse.tile as tile
    from concourse import bacc, mybir
    from concourse.masks import make_identity

    fp32 = mybir.dt.float32
    bf16 = mybir.dt.bfloat16
    AF = mybir.ActivationFunctionType
    ALU = mybir.AluOpType

    nc = bacc.Bacc(None, target_bir_lowering=False)

    names = {}

    with tile.TileContext(nc) as tc:
        with tc.tile_pool(name="dram", bufs=1, space="DRAM") as dram:
            d_query = dram.tile([Q, D], bf16, kind="ExternalInput")
            d_keyT = dram.tile([D, KV], bf16, kind="ExternalInput")
            d_valT = dram.tile([D, KV], bf16, kind="ExternalInput")
            d_keepT = dram.tile([KV, Q], bf16, kind="ExternalInput")
            d_wqT = dram.tile([D, D], bf16, kind="ExternalInput")
            d_wkT = dram.tile([D, D], bf16, kind="ExternalInput")
            d_wvT = dram.tile([D, D], bf16, kind="ExternalInput")
            d_woT = dram.tile([D, D], bf16, kind="ExternalInput")
            d_sel = dram.tile([2, 128, 128], bf16, kind="ExternalInput")
            d_bq = dram.tile([128, 6], fp32, kind="ExternalInput")
            d_cvec = dram.tile([1, D], fp32, kind="ExternalInput")
            d_out = dram.tile([Q, D], bf16, kind="ExternalOutput")
            d_rscratch = dram.tile([H, Q], fp32)

            names = dict(
                query=d_query.name, keyT=d_keyT.name, valT=d_valT.name,
                keepT=d_keepT.name, wqT=d_wqT.name, wkT=d_wkT.name,
                wvT=d_wvT.name, woT=d_woT.name, bq=d_bq.name, sel=d_sel.name,
                cvec=d_cvec.name, out=d_out.name,
            )

            # ---------------- persistent SBUF ----------------
            persist_cm = tc.tile_pool(name="persist", bufs=1)
            persist = persist_cm.__enter__()
            ident = persist.tile([128, 128], bf16)
            make_identity(nc, ident)
            ones_col = persist.tile([128, 1], bf16)
            nc.vector.memset(ones_col, 1.0)
            bq_sb = persist.tile([128, 6], fp32)
            nc.sync.dma_start(out=bq_sb, in_=d_bq[:])
            cvec_sb = persist.tile([128, D], fp32)
            cv_ap = d_cvec[:]
            nc.gpsimd.dma_start(
                out=cvec_sb,
                in_=bass.AP(tensor=cv_ap.tensor, offset=cv_ap.offset,
                            ap=[[0, 128]] + list(cv_ap.ap[1:])),
            )

            sel_sb = persist.tile([128, 2, 128], bf16)
            nc.sync.dma_start(out=sel_sb, in_=d_sel[:].rearrange("s p k -> p s k"))
            qT_sb = [persist.tile([128, Q], bf16, tag=f"qT{i}", name=f"qT{i}") for i in range(6)]
            oT_sb = [persist.tile([128, Q], bf16, tag=f"oT{i}", name=f"oT{i}") for i in range(6)]
            v_sb = [persist.tile([128, D], bf16, tag=f"v{i}", name=f"v{i}") for i in range(NSUB)]
            keep_sb = [persist.tile([128, Q], bf16, tag=f"kp{i}", name=f"kp{i}") for i in range(NSUB)]

            # weights (wq is scoped to phase A below)
            wpool_cm = tc.tile_pool(name="weights", bufs=1)
            wpool = wpool_cm.__enter__()
            wkT_sb = [wpool.tile([128, D], bf16, tag=f"wk{i}", name=f"wk{i}") for i in range(6)]
            wvT_sb = [wpool.tile([128, D], bf16, tag=f"wv{i}", name=f"wv{i}") for i in range(6)]
            woT_sb = [wpool.tile([128, D], bf16, tag=f"wo{i}", name=f"wo{i}") for i in range(6)]

            # kT pool: 6 logical tiles over 4 buffers (pair for group g+1 is
            # produced while pair for group g is consumed; pair g-1 is dead).
            kpool_cm = tc.tile_pool(name="kT", bufs=4)
            kpool = kpool_cm.__enter__()
            kt_tiles = [kpool.tile([128, KV], bf16, tag="kT", name=f"kT{o}")
                        for o in range(2)]

            # staging + eviction pools for the projections
            kst_cm = tc.tile_pool(name="kst", bufs=3)
            kst_pool = kst_cm.__enter__()
            vst_cm = tc.tile_pool(name="vst", bufs=2)
            vst_pool = vst_cm.__enter__()
            pj_cm = tc.tile_pool(name="ps_pj", bufs=1, space="PSUM")
            ps_pj = pj_cm.__enter__()

            # ---------- projection emitters (split DMAs across queues) ----------
            def stage_k(c):
                kst = kst_pool.tile([128, 6, 512], bf16, tag="kst")
                for i in range(6):
                    nc.sync.dma_start(
                        out=kst[:, i, :],
                        in_=d_keyT[128 * i:128 * (i + 1), 512 * c:512 * (c + 1)],
                    )
                return kst

            def emit_kproj(kst, c, o):
                # kT[o][:, 512c:512c+512] = sum_i WkT[i, o-block].T @ keyT[i, chunk]
                ps = ps_pj.tile([128, 512], fp32, tag="pj")
                for i in range(6):
                    nc.tensor.matmul(
                        out=ps[:], lhsT=wkT_sb[i][:, 128 * o:128 * (o + 1)],
                        rhs=kst[:, i, :], start=(i == 0), stop=(i == 5),
                    )
                nc.vector.tensor_copy(
                    out=kt_tiles[o][:, 512 * c:512 * (c + 1)], in_=ps[:])

            def stage_v(c):
                vst = vst_pool.tile([128, 6, 512], bf16, tag="vst")
                for i in range(6):
                    nc.sync.dma_start(
                        out=vst[:, i, :],
                        in_=d_valT[128 * i:128 * (i + 1), 512 * c:512 * (c + 1)],
                    )
                return vst

            def emit_vproj(vst, c, sub, half):
                # v[4c+sub][:, 384*half:...] = sum_i valT[i, t128].T @ WvT[i, half]
                s = 4 * c + sub
                ps = ps_pj.tile([128, 512], fp32, tag="pj")
                for i in range(6):
                    nc.tensor.matmul(
                        out=ps[:, 0:384],
                        lhsT=vst[:, i, 128 * sub:128 * (sub + 1)],
                        rhs=wvT_sb[i][:, 384 * half:384 * (half + 1)],
                        start=(i == 0), stop=(i == 5),
                    )
                nc.vector.tensor_copy(
                    out=v_sb[s][:, 384 * half:384 * (half + 1)], in_=ps[:, 0:384])

            # =============== Phase A: LN(query) -> z.T -> qT ===============
            scopeA = nc.named_scope("phaseA_ln_q"); scopeA.__enter__()
            with (
                tc.tile_pool(name="phA", bufs=2) as phA,
                tc.tile_pool(name="phA1", bufs=1) as phA1,
                tc.tile_pool(name="phA_ps", bufs=3, space="PSUM") as phA_ps,
                tc.tile_pool(name="phA_ps2", bufs=2, space="PSUM") as phA_ps2,
            ):
                # DMA priority order: query (feeds the serial LN->qT chain),
                # then wk + first K/V staging chunks, then the rest.
                q_nat = d_query[:].rearrange("(c p) d -> p c d", p=128)
                x_ts = [phA1.tile([128, D], bf16, tag=f"lnx{c}", name=f"lnx{c}") for c in range(4)]
                for c in range(4):
                    nc.sync.dma_start(out=x_ts[c][:, 0:384], in_=q_nat[:, c, 0:384])
                    nc.sync.dma_start(out=x_ts[c][:, 384:768], in_=q_nat[:, c, 384:768])
                for i in range(6):
                    nc.sync.dma_start(out=wkT_sb[i], in_=d_wkT[128 * i:128 * (i + 1), :])
                proj_state = {"budget": 0.0}
                kst0 = stage_k(0)
                vst0 = stage_v(0)
                stages_k = {0: kst0}
                stages_v = {0: vst0}
                wqT_sb = [phA1.tile([128, D], bf16, tag=f"wq{i}", name=f"wq{i}") for i in range(6)]
                for i in range(6):
                    nc.sync.dma_start(out=wqT_sb[i], in_=d_wqT[128 * i:128 * (i + 1), :])
                    nc.sync.dma_start(out=wvT_sb[i], in_=d_wvT[128 * i:128 * (i + 1), :])
                stages_k[1] = stage_k(1)
                stages_v[1] = stage_v(1)
                for s in range(NSUB):
                    nc.sync.dma_start(out=keep_sb[s], in_=d_keepT[128 * s:128 * (s + 1), :])
                for i in range(6):
                    nc.sync.dma_start(out=woT_sb[i], in_=d_woT[128 * i:128 * (i + 1), :])
                eps_sb = phA1.tile([128, 1], fp32)
                nc.vector.memset(eps_sb, 1e-5)

                z_bf = x_ts  # LN is applied in place on the staged query tiles
                for c in range(4):
                    x_t = x_ts[c]
                    # bn_stats needs free dim <= 512; use 3 subgroups of 256
                    stats = phA.tile([128, 3, 6], fp32, tag="lnst")
                    xg = x_t[:].rearrange("p (s d) -> p s d", s=3)
                    for s in range(3):
                        nc.vector.bn_stats(out=stats[:, s, :], in_=xg[:, s, :])
                    mv = phA.tile([128, 2], fp32, tag="lnmv")
                    nc.vector.bn_aggr(out=mv, in_=stats[:])
                    rstd = phA.tile([128, 1], fp32, tag="lnrs")
                    nc.scalar.activation(out=rstd, in_=mv[:, 1:2], func=AF.Sqrt,
                                         bias=eps_sb[:], scale=1.0)
                    nc.vector.reciprocal(out=rstd, in_=rstd)
                    nc.vector.tensor_scalar(
                        out=z_bf[c][:], in0=x_t[:], scalar1=mv[:, 0:1],
                        scalar2=rstd[:], op0=ALU.subtract, op1=ALU.mult,
                    )
                # transpose z -> zT (6 tiles [128, 512]); c-major so each
                # z tile's transposes start as soon as its LN completes
                zT = [phA1.tile([128, Q], bf16, tag=f"zT{i}", name=f"zT{i}") for i in range(6)]
                zps = [phA_ps.tile([128, 2, Q], bf16, tag="zTps", name=f"zps{i}") for i in range(3)]
                for c in range(4):
                    for i in range(6):
                        nc.tensor.transpose(
                            out=zps[i // 2][:, i % 2, 128 * c:128 * (c + 1)],
                            in_=z_bf[c][:, 128 * i:128 * (i + 1)],
                            identity=ident[:],
                        )
                for i in range(6):
                    nc.vector.tensor_copy(out=zT[i][:], in_=zps[i // 2][:, i % 2, :])
                # qT[o,:] = sum_i WqT[i, o-block].T @ zT[i]; group 0's two
                # o-blocks first, then the first kT chunk, then the rest
                def emit_qt(o):
                    ps = phA_ps2.tile([128, Q], fp32, tag="qps", name="qps")
                    for i in range(6):
                        nc.tensor.matmul(
                            out=ps[:], lhsT=wqT_sb[i][:, 128 * o:128 * (o + 1)],
                            rhs=zT[i][:], start=(i == 0), stop=(i == 5),
                        )
                    nc.vector.tensor_scalar(
                        out=qT_sb[o][:], in0=ps[:], scalar1=bq_sb[:, o:o + 1],
                        scalar2=None, op0=ALU.add,
                    )
                for o in range(6):
                    emit_qt(o)
            scopeA.__exit__(None, None, None)

            # =============== Prologue: kT chunk 0 of pair 0, first V chunk ===============
            scopeB = nc.named_scope("phaseB_prologue"); scopeB.__enter__()
            emit_kproj(kst0, 0, 0)
            emit_kproj(kst0, 0, 1)

            scopeB.__exit__(None, None, None)

            # deferred projection work, interleaved into attention loops.
            # Staging DMAs are issued one wave ahead of their consumers.
            from collections import deque
            pend_g = [deque(), deque(), deque()]
            kt_tiles.append(kpool.tile([128, KV], bf16, tag="kT", name="kT2"))
            kt_tiles.append(kpool.tile([128, KV], bf16, tag="kT", name="kT3"))
            kt_tiles.append(kpool.tile([128, KV], bf16, tag="kT", name="kT4"))
            kt_tiles.append(kpool.tile([128, KV], bf16, tag="kT", name="kT5"))
            # group 0: V chunk 0 first, then K01/V chunks 1..7; K23 chunk 0
            for sub in range(4):
                for half in range(2):
                    pend_g[0].append((emit_vproj, (None, 0, sub, half)))
            for c in range(1, NT):
                pend_g[0].append((emit_kproj, (None, c, 0)))
                pend_g[0].append((emit_kproj, (None, c, 1)))
                pend_g[0].append((stage_k, (c + 1 if c + 1 < NT else 0,)))
                for sub in range(4):
                    for half in range(2):
                        pend_g[0].append((emit_vproj, (None, c, sub, half)))
                if c + 1 < NT:
                    pend_g[0].append((stage_v, (c + 1,)))
            pend_g[0].append((stage_k, (1,)))
            pend_g[0].append((emit_kproj, (None, 0, 2)))
            pend_g[0].append((emit_kproj, (None, 0, 3)))
            # group 1: single pass over chunks 1..7 for all four o-blocks
            # (kT4/kT5 reuse kT0/kT1's buffers, free once group 0 is done),
            # then the re-staged chunk 0 for o=4,5.
            for c in range(1, NT):
                pend_g[1].append((emit_kproj, (None, c, 2)))
                pend_g[1].append((emit_kproj, (None, c, 3)))
                pend_g[1].append((emit_kproj, (None, c, 4)))
                pend_g[1].append((emit_kproj, (None, c, 5)))
                pend_g[1].append((stage_k, (c + 1 if c + 1 < NT else 0,)))
            pend_g[1].append((emit_kproj, (None, 0, 4)))
            pend_g[1].append((emit_kproj, (None, 0, 5)))

            # group 2: partial out-projection (i=0..3 contributions + cvec)
            eacc_holder = {}

            def emit_epartial(tc_i, half):
                ps = ps_pj.tile([128, 512], fp32, tag="pj")
                for i in range(4):
                    nc.tensor.matmul(
                        out=ps[:, 0:384],
                        lhsT=oT_sb[i][:, 128 * tc_i:128 * (tc_i + 1)],
                        rhs=woT_sb[i][:, 384 * half:384 * (half + 1)],
                        start=(i == 0), stop=(i == 3),
                    )
                nc.vector.tensor_add(
                    out=eacc_holder["t"][2 * tc_i + half][:, 0:384], in0=ps[:, 0:384],
                    in1=cvec_sb[:, 384 * half:384 * (half + 1)])

            for tc_i in range(4):
                for half in range(2):
                    pend_g[2].append((emit_epartial, (tc_i, half)))

            # per-iteration thunk quotas
            QUOTA = [len(pend_g[0]) / 27.0, len(pend_g[1]) / 26.0,
                     len(pend_g[2]) / 20.0]

            # =============== Attention (+ interleaved projections) ===============
            scopeD = nc.named_scope("phaseD_attn"); scopeD.__enter__()
            with (
                tc.tile_pool(name="phD", bufs=7) as phD,
                tc.tile_pool(name="phD1", bufs=1) as phD1,
                tc.tile_pool(name="phD_s", bufs=2, space="PSUM") as phD_s,
                tc.tile_pool(name="phD_o", bufs=2, space="PSUM") as phD_o,
                tc.tile_pool(name="phD_r", bufs=1, space="PSUM") as phD_r,
            ):
                for g in range(NG):
                    proj_state["budget"] = 2.5 if g == 1 else (2.0 if g == 0 else 0.0)
                    if g == 2:
                        # reuse the first 8 keep tiles (dead past group-2 use)
                        # as out-projection partial accumulators
                        eacc_holder["t"] = keep_sb[:8]
                    o01 = phD_o.tile([128, Q], fp32, tag="opair")
                    o23 = phD_o.tile([128, Q], fp32, tag="opair")
                    opair = (o01, o23)
                    r_ps = phD_r.tile([128, Q], fp32, tag="rps")
                    nc.vector.memset(r_ps, 1.0)
                    kts = (kt_tiles[2 * g], kt_tiles[2 * g + 1])
                    qts = (qT_sb[2 * g], qT_sb[2 * g + 1])

                    def emit_half(s, hh):
                        # scores + exp + mask for heads (4g+2hh, 4g+2hh+1)
                        sl = slice(128 * s, 128 * (s + 1))
                        s2 = phD_s.tile([128, 2, Q], fp32, tag="s4")
                        kt, qt = kts[hh], qts[hh]
                        for j in range(2):
                            nc.tensor.matmul(
                                out=s2[:, j, :],
                                lhsT=kt[64 * j:64 * (j + 1), sl],
                                rhs=qt[64 * j:64 * (j + 1), :],
                                start=True, stop=True,
                                tile_position=(64 * j, 0),
                            )
                        e2 = phD.tile([128, 2, Q], bf16, tag="e4")
                        nc.scalar.activation(out=e2[:], in_=s2[:], func=AF.Exp)
                        kap = keep_sb[s][:]
                        nc.vector.tensor_mul(
                            e2[:], e2[:],
                            bass.AP(tensor=kap.tensor, offset=kap.offset,
                                    ap=[kap.ap[0], [0, 2]] + list(kap.ap[1:])),
                        )
                        return e2

                    def emit_pv(s, epair):
                        # PV pairs first (col-tile concurrency), then the four
                        # row-sum matmuls together (4-way col-tile concurrency)
                        for hh in range(2):
                            e2 = epair[hh]
                            for j in range(2):
                                h = 2 * hh + j
                                nc.tensor.matmul(
                                    out=opair[hh][64 * j:64 * (j + 1), :],
                                    lhsT=v_sb[s][:, 256 * g + 64 * h:256 * g + 64 * (h + 1)],
                                    rhs=e2[:, j, :],
                                    start=(s == 0), stop=(s == NSUB - 1),
                                    tile_position=(0, 64 * j),
                                    skip_group_check=True,
                                )
                        for h in range(4):
                            nc.tensor.matmul(
                                out=r_ps[32 * h:32 * h + 1, :],
                                lhsT=ones_col[:],
                                rhs=epair[h // 2][:, h % 2, :],
                                start=(s == 0), stop=(s == NSUB - 1),
                                tile_position=(0, 32 * h),
                                skip_group_check=True,
                            )

                    def pop_thunks(state, g=g):
                        state["budget"] += QUOTA[g]
                        dq = pend_g[g]
                        while state["budget"] >= 1.0 and dq:
                            fn, args = dq.popleft()
                            if fn is stage_k:
                                stages_k[args[0]] = fn(*args)
                            elif fn is stage_v:
                                stages_v[args[0]] = fn(*args)
                            elif fn is emit_kproj:
                                fn(stages_k[args[1]], *args[1:])
                            elif fn is emit_vproj:
                                fn(stages_v[args[1]], *args[1:])
                            else:
                                fn(*args)
                            state["budget"] -= 1.0

                    e_q = []
                    for s in range(NSUB):
                        eA = emit_half(s, 0)
                        eB = emit_half(s, 1)
                        e_q.append((eA, eB))
                        pop_thunks(proj_state)
                        if s >= 2:
                            emit_pv(s - 2, e_q[s - 2])
                    emit_pv(NSUB - 2, e_q[NSUB - 2])
                    emit_pv(NSUB - 1, e_q[NSUB - 1])
                    # drain any leftover thunks for this group
                    proj_state["budget"] = 1e9
                    pop_thunks(proj_state)
                    proj_state["budget"] = 0.0

                    # r -> 1/r, then broadcast 1/r rows to 64-row blocks with
                    # a selector-mask matmul on the PE (no DRAM bounce):
                    # rb[p, q] = sum_c sel[c, p] * rinv[c, q]
                    rinv = phD1.tile([128, Q], fp32, tag="rinv")
                    nc.vector.reciprocal_approx_fast(out=rinv[:], in_=r_ps[:])
                    rinv_bf = phD1.tile([128, Q], bf16, tag="rinvb")
                    nc.vector.tensor_copy(out=rinv_bf[:], in_=rinv[:])
                    rb_ps = phD_s.tile([128, 2, Q], fp32, tag="s4")
                    for hh in range(2):
                        nc.tensor.matmul(
                            out=rb_ps[:, hh, :], lhsT=sel_sb[:, hh, :],
                            rhs=rinv_bf[:], start=True, stop=True,
                        )
                    nc.vector.tensor_copy(out=oT_sb[2 * g][:], in_=o01[:])
                    nc.vector.tensor_copy(out=oT_sb[2 * g + 1][:], in_=o23[:])
                    nc.vector.tensor_mul(oT_sb[2 * g][:], oT_sb[2 * g][:], rb_ps[:, 0, :])
                    nc.vector.tensor_mul(oT_sb[2 * g + 1][:], oT_sb[2 * g + 1][:], rb_ps[:, 1, :])

                # ---- Phase E tail: add oT[4:6] contributions to partials ----
                out_nat = d_out[:].rearrange("(c p) d -> p c d", p=128)
                for tchunk_i in range(4):
                    ob = phD1.tile([128, D], bf16, tag=f"ob{tchunk_i % 2}", name=f"ob{tchunk_i % 2}")
                    for half in range(2):
                        ps = phD_s.tile([128, 2, Q], fp32, tag="s4")
                        for i in range(4, 6):
                            nc.tensor.matmul(
                                out=ps[:, 0, 0:384],
                                lhsT=oT_sb[i][:, 128 * tchunk_i:128 * (tchunk_i + 1)],
                                rhs=woT_sb[i][:, 384 * half:384 * (half + 1)],
                                start=(i == 4), stop=(i == 5),
                            )
                        nc.vector.tensor_add(
                            out=ob[:, 384 * half:384 * (half + 1)], in0=ps[:, 0, 0:384],
                            in1=eacc_holder["t"][2 * tchunk_i + half][:, 0:384])
                    for qd in range(2):
                        nc.sync.dma_start(out=out_nat[:, tchunk_i, 384 * qd:384 * (qd + 1)],
                                          in_=ob[:, 384 * qd:384 * (qd + 1)])

            scopeD.__exit__(None, None, None)
            pj_cm.__exit__(None, None, None)
            vst_cm.__exit__(None, None, None)
            kst_cm.__exit__(None, None, None)
            kpool_cm.__exit__(None, None, None)
            wpool_cm.__exit__(None, None, None)
            persist_cm.__exit__(None, None, None)

    nc.compile()
    return nc, names


def kernel(**inputs):
    from concourse.bass_utils import run_bass_kernel_spmd

    nc, names, in_maps = _make_in_maps(inputs)
    res = run_bass_kernel_spmd(nc, in_maps, list(range(B)))
    out = np.stack([np.asarray(r[names["out"]], dtype=np.float32)
                    for r in res.results], axis=0)
    return out


def _make_in_maps(inputs):
    nc, names = _build()
    query = _f32(inputs["query"])
    key = _f32(inputs["key"])
    value = _f32(inputs["value"])
    mask = np.asarray(inputs["attention_mask"], dtype=np.int32)
    Wq = _f32(inputs["Wq"]); bq = _f32(inputs["bq"])
    Wk = _f32(inputs["Wk"])
    Wv = _f32(inputs["Wv"]); bv = _f32(inputs["bv"])
    Wo = _f32(inputs["Wo"]); bo = _f32(inputs["bo"])
    ln_g = _f32(inputs["ln_g"]); ln_b = _f32(inputs["ln_b"])
    scale = 1.0 / np.sqrt(DH)
    wqT = _bf16((Wq * ln_g[None, :] * scale).T)
    bq_eff = (ln_b @ Wq.T + bq) * scale
    bq_arr = _f32(bq_eff.reshape(6, 128).T)
    wkT = _bf16(Wk.T)
    wvT = _bf16(Wv.T)
    woT = _bf16(Wo.T)
    cvec = _f32((bv @ Wo.T + bo).reshape(1, D))
    sel = np.zeros((2, 128, 128), np.float32)
    sel[0, 0, 0:64] = 1.0
    sel[0, 32, 64:128] = 1.0
    sel[1, 64, 0:64] = 1.0
    sel[1, 96, 64:128] = 1.0
    keep = (1 - mask).astype(np.float32)
    in_maps = []
    for b in range(B):
        in_maps.append({
            names["query"]: _bf16(query[b]),
            names["keyT"]: _bf16(key[b].T),
            names["valT"]: _bf16(value[b].T),
            names["keepT"]: _bf16(keep[b].T),
            names["wqT"]: wqT, names["wkT"]: wkT, names["wvT"]: wvT,
            names["woT"]: woT, names["bq"]: bq_arr, names["cvec"]: cvec,
            names["sel"]: _bf16(sel),
        })
    return nc, names, in_maps


def run_traced(**inputs):
    """Run with tracing enabled; returns exec_time_ns (or None)."""
    from concourse.bass_utils import run_bass_kernel_spmd
    nc, names, in_maps = _make_in_maps(inputs)
    res = run_bass_kernel_spmd(nc, in_maps, list(range(B)), trace=True)
    if res.instructions_and_trace is not None:
        print("trace:", res.instructions_and_trace[1])
    print("mean exec ns:", res.mean_exec_time_ns, "max core:", res.max_exec_time_core_id)
    if res.per_core_scope_times:
        for scope, cores in sorted(res.per_core_scope_times.items()):
            for cid, dur in cores.items():
                print(f"  scope {scope}: core{cid} {dur} ns")
    return res.exec_time_ns


if __name__ == "__main__":
    rng = np.random.default_rng(0)
    dummy = {
        "query": rng.standard_normal((B, Q, D), dtype=np.float32),
        "key": rng.standard_normal((B, KV, D), dtype=np.float32),
        "value": rng.standard_normal((B, KV, D), dtype=np.float32),
        "attention_mask": rng.integers(0, 2, (B, Q, KV)).astype(np.int32),
        "Wq": rng.standard_normal((D, D), dtype=np.float32) / 27.7,
        "bq": np.zeros(D, np.float32),
        "Wk": rng.standard_normal((D, D), dtype=np.float32) / 27.7,
        "bk": np.zeros(D, np.float32),
        "Wv": rng.standard_normal((D, D), dtype=np.float32) / 27.7,
        "bv": np.zeros(D, np.float32),
        "Wo": rng.standard_normal((D, D), dtype=np.float32) / 27.7,
        "bo": np.zeros(D, np.float32),
        "ln_g": np.ones(D, np.float32),
        "ln_b": np.zeros(D, np.float32),
    }
    out = kernel(**dummy)
    print("out", out.shape, out.dtype, float(np.abs(out).mean()))
